# revision 25
# baseline (speedup 1.0000x reference)
"""Causal self-attention, tensor-parallel over heads across 8 NeuronCores.

Reference computation (per problem):
    qkv = x @ w_attn + b_attn ; split q,k,v ; per-head causal softmax attention
    y = att @ v ; out = y @ w_proj + b_proj
Shapes: x [4, 2048, 1024], H=16 heads, head_size=64.

Sharding: 2 heads per core (tensor parallel). Each core computes
    qkv for its heads, causal attention, and a partial y @ w_proj[rows].
Host sums the 8 partial outputs and adds b_proj (the TP all-reduce,
done host-side during the gather step).

Per-core kernel v3 (all matmuls bf16 with fp32 PSUM accumulate):
  - x passed pre-transposed (xT [C, B*T]); q pre-scaled by 1/sqrt(hs).
  - Software-pipelined schedule: batch b+1's qkv projection + v
    transposition is chopped into small quanta (a Python generator) and
    interleaved into batch b's attention j-tile loop, so TensorE never
    waits for ScalarE's exp backlog.
  - Phase 2 per (b, qg): S^T j-tiles via K=64 matmuls TWO tiles ahead
    of the attention-value matmuls, exp on ScalarE over both heads'
    PSUM banks, single 0/1 mask multiply covering both heads on the
    diagonal tiles, y^T[65, 512] accumulated with a ones-column
    denominator row.
  - Normalize: DVE reciprocal of denom row -> f32r, GpSimd
    partition_broadcast to 64 partitions, DVE multiply into packed
    yp [128, NT] (h0 rows 0-63, h1 rows 64-127, partition-shifted).
  - Phase 3: K=128 matmuls (both heads at once): out[tok, :] =
    yp.T @ w_proj_local; evictions on ScalarE (A/B-tested best).
"""

import math
import numpy as np

try:
    import concourse.bass as bass
except ImportError:  # pragma: no cover
    import sys

    sys.path.insert(0, "/opt/trn_rl_repo")
    import concourse.bass as bass

import ml_dtypes
import concourse.mybir as mybir
import concourse.tile as tile
from concourse import bacc
from concourse.bass_utils import run_bass_kernel_spmd
from concourse.masks import make_identity

BF16 = mybir.dt.bfloat16
F32 = mybir.dt.float32
F32R = mybir.dt.float32r

NCORES = 8

Exp = mybir.ActivationFunctionType.Exp
Identity = mybir.ActivationFunctionType.Identity
Copy = mybir.ActivationFunctionType.Copy


def build_nc(B=4, T=2048, C=1024, H=16, bass_kwargs=None, repeats=1,
             qk_act=False, ob_act_mod=1, p3_delay=1, ob_act_mod_tail=None,
             p3_hold=None, pd=3, fp8_qk=False, mask_pool=False,
             vt_dma=False):
    HS = C // H          # 64 head size
    HPC = H // NCORES    # 2 heads per core
    DC = HPC * HS        # 128 local channels
    NT = B * T           # tokens
    CK = C // 128        # contraction chunks for qkv
    QG = 512             # q-group width
    NQG = T // QG        # q-groups per batch
    NJT = T // 128       # 128-wide key tiles per batch
    JT_PER_QG = QG // 128

    assert DC == 128 and HPC == 2 and NQG == 4

    kw = dict(target_bir_lowering=False, debug=False)
    kw.update(bass_kwargs or {})
    nc = bacc.Bacc("TRN2", **kw)
    xT = nc.dram_tensor("xT", [C, NT], BF16, kind="ExternalInput")
    w_qkv = nc.dram_tensor("w_qkv", [C, 3 * DC], BF16, kind="ExternalInput")
    b_qkv = nc.dram_tensor("b_qkv", [3 * DC], F32, kind="ExternalInput")
    w_proj = nc.dram_tensor("w_proj", [DC, C], BF16, kind="ExternalInput")
    outp = nc.dram_tensor("outp", [NT, C], BF16, kind="ExternalOutput")
    if fp8_qk:
        F8 = mybir.dt.float8e4
        xT8 = nc.dram_tensor("xT8", [C, NT], F8, kind="ExternalInput")
        w_qk8 = nc.dram_tensor("w_qk8", [C, 2 * DC], F8, kind="ExternalInput")

    with tile.TileContext(nc) as tc, tc.tile_pool(name="singles", bufs=1) as sg:
        # ---- persistent SBUF ----
        qT_sb = sg.tile([128, NT], BF16)   # rows 0-63 h0, 64-127 h1
        kT_sb = sg.tile([128, NT], BF16)
        # v in natural layout, one [128, 65] tile per (b, h, j-tile);
        # col 64 is the ones column (softmax denominator trick)
        v_sb = sg.tile([128, B, HPC, NJT, 65], BF16)
        yp_sb = sg.tile([128, NT], BF16)   # packed: h0 rows 0-63, h1 64-127
        w_sb = sg.tile([128, CK, 3 * DC], BF16)
        wp_sb = sg.tile([128, C], BF16)    # both heads' w_proj rows
        bias_sb = sg.tile([128, 3], F32)
        masks_sb = sg.tile([128, JT_PER_QG, HPC, QG], BF16)
        ident_sb = sg.tile([128, 128], BF16)

        if fp8_qk:
            w8_sb = sg.tile([128, CK, 2 * DC], mybir.dt.float8e4)

        # ---- setup, ordered by first use (w chunks -> bias -> w_proj) ----
        w_r = w_qkv.rearrange("(ck p) m -> p ck m", p=128)
        ccs = [2] if fp8_qk else range(3)
        if fp8_qk:
            w8_r = w_qk8.rearrange("(ck p) m -> p ck m", p=128)
            nc.sync.dma_start(out=w8_sb, in_=w8_r)
        for cc in ccs:
            nc.sync.dma_start(
                out=w_sb[:, :, 128 * cc : 128 * cc + 128],
                in_=w_r[:, :, 128 * cc : 128 * cc + 128],
            )
        nc.sync.dma_start(
            out=bias_sb, in_=b_qkv.rearrange("(c p) -> p c", p=128)
        )
        nc.scalar.dma_start(out=wp_sb, in_=w_proj[:, :])
        make_identity(nc, ident_sb)
        # ones column of every v tile, one strided memset
        nc.vector.memset(v_sb[:, :, :, :, 64:65], 1.0)
        # mask[s][j, q] = 1.0 if q >= 128*s + j else 0  (causal, diag tiles)
        if not mask_pool:
            for s in range(JT_PER_QG):
                for h in range(HPC):
                    nc.gpsimd.memset(masks_sb[:, s, h, :], 1.0)
                    nc.gpsimd.affine_select(
                        out=masks_sb[:, s, h, :],
                        in_=masks_sb[:, s, h, :],
                        compare_op=mybir.AluOpType.is_ge,
                        fill=0.0,
                        base=-128 * s,
                        pattern=[[1, QG]],
                        channel_multiplier=-1,
                    )

        with (
            tc.tile_pool(name="xt_pool", bufs=4) as xt_pool,
            tc.tile_pool(name="xt8_pool", bufs=4) as xt8_pool,
            tc.tile_pool(name="vt_pool", bufs=2) as vt_pool,
            tc.tile_pool(name="pt_pool", bufs=4) as pt_pool,
            tc.tile_pool(name="rd_pool", bufs=2) as rd_pool,
            tc.tile_pool(name="bc_pool", bufs=2) as bc_pool,
            tc.tile_pool(name="ob_pool", bufs=4) as ob_pool,
            tc.tile_pool(name="ps_stage", bufs=2, space="PSUM") as ps_stage,
            tc.tile_pool(name="ps_yt", bufs=2, space="PSUM") as ps_yt,
            tc.tile_pool(name="ps_misc", bufs=2, space="PSUM") as ps_misc,
        ):
            xT_r = xT.rearrange("(ck p) n -> p ck n", p=128)
            if fp8_qk:
                xT8_r = xT8.rearrange("(ck p) n -> p ck n", p=128)
                F8 = mybir.dt.float8e4
                DR = mybir.MatmulPerfMode.DoubleRow

            done_groups = {}

            def gen_p1(b):
                """Generator: qkv + v-transpose for batch b, in small PE
                quanta (one yield per ~2-4 matmuls). Updates done_groups[b]
                after each 512-token group completes. Batch 0's x loads go
                out on the (startup-idle) DVE queue so they overlap the
                weight DMAs on the sync queue."""
                done_groups[b] = 0
                xts = []
                xt8s = []
                for g in range(NQG):
                    off = b * T + QG * g
                    xt = xt_pool.tile([128, CK, 512], BF16, tag="xt")
                    # batch 0 group 0 rides the startup-idle ACT queue so it
                    # overlaps the weight DMAs on the sync queue
                    dma_eng = nc.scalar if (b == 0 and g == 0) else nc.sync
                    dma_eng.dma_start(
                        out=xt, in_=xT_r[:, :, off : off + 512]
                    )
                    xts.append(xt)
                    if fp8_qk:
                        xt8 = xt8_pool.tile([128, CK, 512], F8, tag="xt8")
                        dma_eng.dma_start(
                            out=xt8, in_=xT8_r[:, :, off : off + 512]
                        )
                        xt8s.append(xt8)
                yield
                for g in range(NQG):
                    off = b * T + QG * g
                    vtmp = vt_pool.tile([128, 512], BF16, tag="vt")
                    for cc in range(3):
                        qkv_ps = ps_misc.tile([128, 512], F32, tag="misc")
                        if fp8_qk and cc < 2:
                            for c2 in range(CK // 2):
                                nc.tensor.matmul(
                                    qkv_ps,
                                    w8_sb[:, 2 * c2 : 2 * c2 + 2,
                                          128 * cc : 128 * cc + 128],
                                    xt8s[g][:, 2 * c2 : 2 * c2 + 2, :],
                                    start=(c2 == 0),
                                    stop=(c2 == CK // 2 - 1),
                                    perf_mode=DR,
                                )
                                if c2 == 1:
                                    yield
                        else:
                            for k in range(CK):
                                nc.tensor.matmul(
                                    qkv_ps,
                                    w_sb[:, k, 128 * cc : 128 * cc + 128],
                                    xts[g][:, k, :],
                                    start=(k == 0),
                                    stop=(k == CK - 1),
                                )
                                if k == 3:
                                    yield
                        if cc < 2:
                            dest = qT_sb if cc == 0 else kT_sb
                            if qk_act:
                                nc.scalar.activation(
                                    out=dest[:, off : off + 512],
                                    in_=qkv_ps,
                                    func=Identity,
                                    bias=bias_sb[:, cc : cc + 1],
                                )
                            else:
                                nc.vector.tensor_scalar_add(
                                    out=dest[:, off : off + 512],
                                    in0=qkv_ps,
                                    scalar1=bias_sb[:, cc : cc + 1],
                                )
                        else:
                            nc.vector.tensor_scalar_add(
                                out=vtmp,
                                in0=qkv_ps,
                                scalar1=bias_sb[:, 2:3],
                            )
                        yield
                    for s in range(JT_PER_QG):
                        jt = JT_PER_QG * g + s
                        for h in range(HPC):
                            if vt_dma:
                                # HWDGE 16-bit transpose: SBUF->SBUF, no
                                # PE transpose + no DVE PSUM eviction
                                nc.sync.dma_start_transpose(
                                    out=v_sb[:, b, h, jt, 0:64],
                                    in_=vtmp[64 * h : 64 * h + 64,
                                             128 * s : 128 * s + 128],
                                )
                                continue
                            vt_ps = ps_misc.tile([128, 64], BF16, tag="misc")
                            nc.tensor.transpose(
                                vt_ps,
                                vtmp[64 * h : 64 * h + 64,
                                     128 * s : 128 * s + 128],
                                ident_sb[64 * h : 64 * h + 64,
                                         64 * h : 64 * h + 64],
                            )
                            nc.vector.tensor_copy(
                                out=v_sb[:, b, h, jt, 0:64], in_=vt_ps
                            )
                        yield
                    done_groups[b] = g + 1

            fillers = []

            def fill(n=1):
                for _ in range(n):
                    while fillers:
                        if next(fillers[0], StopIteration) is StopIteration:
                            fillers.pop(0)
                        else:
                            break
                    if not fillers:
                        return

            def fill_until_group(b, g):
                while done_groups.get(b, 0) < g and fillers:
                    fill(1)

            pre_norm_hook = [None]

            def emit_p2(b, qg):
                """Causal attention for (b, q-group); writes yp_sb columns."""
                q0 = b * T + QG * qg
                njt = JT_PER_QG * (qg + 1)
                yts = [
                    ps_yt.tile([65, 512], F32, tag="yt", name=f"yt{h}")
                    for h in range(HPC)
                ]

                def col_lo(jt):
                    # diagonal sub-tile s: columns [0, 128*s) are entirely
                    # non-causal -- skip them in S / exp / mask / y
                    s = jt - JT_PER_QG * qg
                    return 128 * s if s > 0 else 0

                def emit_S(jt):
                    lo = col_lo(jt)
                    st = ps_stage.tile([128, 2, 512], F32, tag="stage")
                    for h in range(HPC):
                        hl = slice(64 * h, 64 * h + 64)
                        nc.tensor.matmul(
                            st[:, h, lo:QG],
                            kT_sb[hl, b * T + 128 * jt : b * T + 128 * jt + 128],
                            qT_sb[hl, q0 + lo : q0 + QG],
                            start=True,
                            stop=True,
                        )
                    pt = pt_pool.tile([128, 2, 512], BF16, tag="pt")
                    # fp8_qk: q,k were computed from host-prescaled (x16)
                    # fp8 weights+biases; fold 1/(16*16) and the 1/sqrt(hs)
                    # attention scale into the exp activation's scale.
                    nc.scalar.activation(
                        out=pt[:, :, lo:QG], in_=st[:, :, lo:QG], func=Exp,
                        scale=(0.125 / 256.0) if fp8_qk else 1.0,
                    )
                    if jt >= JT_PER_QG * qg:  # diagonal tile: causal zeroing
                        s = jt - JT_PER_QG * qg
                        if mask_pool:
                            # keep pt[j, h, q] where (lo+qr) >= 128*s + j;
                            # lo == 128*s, so base = 0. Runs on the
                            # (otherwise idle) Pool engine instead of DVE.
                            nc.gpsimd.affine_select(
                                out=pt[:, :, lo:QG],
                                in_=pt[:, :, lo:QG],
                                compare_op=mybir.AluOpType.is_ge,
                                fill=0.0,
                                base=0,
                                pattern=[[0, HPC], [1, QG - lo]],
                                channel_multiplier=-1,
                            )
                        else:
                            nc.vector.tensor_mul(
                                pt[:, :, lo:QG],
                                pt[:, :, lo:QG],
                                masks_sb[:, s, :, lo:QG],
                            )
                    return pt

                def emit_y(pt, jt):
                    lo = col_lo(jt)
                    for h in range(HPC):
                        nc.tensor.matmul(
                            yts[h][:, lo:QG],
                            v_sb[:, b, h, jt, :],
                            pt[:, h, lo:QG],
                            start=(jt == 0),
                            stop=(jt == njt - 1),
                        )

                pts = {}
                for jt in range(njt):
                    pts[jt] = emit_S(jt)
                    fill(1)
                    if jt >= pd:
                        emit_y(pts.pop(jt - pd), jt - pd)
                        fill(1)
                for jt in range(max(0, njt - pd), njt):
                    emit_y(pts.pop(jt), jt)
                    if jt < njt - 1:
                        fill(1)
                if pre_norm_hook[0] is not None:
                    hook, pre_norm_hook[0] = pre_norm_hook[0], None
                    hook()

                # normalize: yp[:, q] = y[:, q] / den[q]
                for h in range(HPC):
                    rd = rd_pool.tile([1, 512], F32R, tag="rd")
                    with nc.allow_low_precision("f32r denom recip"):
                        nc.vector.reciprocal(rd, yts[h][64:65, :])
                    bc_sb = bc_pool.tile([64, 512], F32R, tag="bc")
                    nc.gpsimd.partition_broadcast(bc_sb, rd)
                    nc.vector.tensor_mul(
                        out=yp_sb[64 * h : 64 * h + 64, q0 : q0 + QG],
                        in0=yts[h][0:64, :],
                        in1=bc_sb,
                    )

            def emit_p3(b, qg, cur_b):
                """out[tok, :] = yp.T @ w_proj for this q-group's tokens.
                cur_b: the batch whose attention loop we're emitting inside
                (controls which engine takes the eviction)."""
                mod = ob_act_mod
                if ob_act_mod_tail is not None and cur_b == B - 1:
                    mod = ob_act_mod_tail
                for i in range(QG // 128):
                    t0 = b * T + QG * qg + 128 * i
                    pr = ps_stage.tile([128, 2, 512], F32, tag="stage")
                    for ns in range(2):
                        nc.tensor.matmul(
                            pr[:, ns, :],
                            yp_sb[:, t0 : t0 + 128],
                            wp_sb[:, 512 * ns : 512 * ns + 512],
                            start=True,
                            stop=True,
                        )
                    ob = ob_pool.tile([128, 1024], BF16, tag="ob")
                    if mod == -1:  # split halves across ACT and DVE
                        nc.scalar.activation(
                            out=ob[:, 0:512], in_=pr[:, 0, :], func=Copy
                        )
                        nc.vector.tensor_copy(
                            out=ob[:, 512:1024], in_=pr[:, 1, :]
                        )
                    elif mod and i % mod == 0:
                        nc.scalar.activation(out=ob, in_=pr, func=Copy)
                    else:
                        nc.vector.tensor_copy(out=ob, in_=pr)
                    nc.sync.dma_start(out=outp[t0 : t0 + 128, :], in_=ob)
                    fill(1)

            for _rep in range(repeats):
                # Progressive prologue: attention for (b, qg) starts as soon
                # as batch b's qkv groups 0..qg are emitted; the rest of the
                # qkv work (and the next batch's) rides along as filler
                # inside the attention loops. P3 runs one q-group late so
                # the normalize chain overlaps the next q-group's S matmuls.
                hold = p3_hold or {}
                p3_pending = []
                fillers.clear()
                done_groups.clear()
                fillers.append(gen_p1(0))
                for b in range(B):
                    if b + 1 < B:
                        fillers.append(gen_p1(b + 1))
                    # last batch: largest q-group first, so the run ends on
                    # the smallest exp backlog
                    qgs = range(NQG) if b + 1 < B else range(NQG - 1, -1, -1)
                    for qg in qgs:
                        fill_until_group(b, qg + 1)
                        emit_p2(b, qg)
                        p3_pending.append((b, qg))
                        while len(p3_pending) > hold.get(b, p3_delay):
                            emit_p3(*p3_pending.pop(0), cur_b=b)
                fill(10**6)
                while p3_pending:
                    emit_p3(*p3_pending.pop(0), cur_b=B - 1)

    nc.compile()
    return nc


_NC_CACHE = {}

# variant knobs used by kernel() (and test.py).
# vt_dma (HWDGE 16-bit DMA transpose for v) measured -10% marginal HW
# time in a reps-65 paired A/B, and passes CoreSim exec bit-for-bit,
# but produces NaNs on REAL hardware (sim/HW divergence in the DMA
# transpose path) -- kept OFF. Do not re-enable without a full
# kernel()-vs-reference check on device.
KERNEL_KWARGS = {}


def _get_nc(shape_key):
    if shape_key not in _NC_CACHE:
        _NC_CACHE[shape_key] = build_nc(*shape_key, **KERNEL_KWARGS)
    return _NC_CACHE[shape_key]


def make_in_maps(x, w_attn, b_attn, w_proj, B, T, C, H, fp8_qk=False):
    HS = C // H
    HPC = H // NCORES
    DC = HPC * HS
    scale = 1.0 / math.sqrt(HS)
    bf = ml_dtypes.bfloat16
    f8 = ml_dtypes.float8_e4m3

    xT = np.ascontiguousarray(
        x.reshape(B * T, C).T.astype(bf)
    )
    # w_attn columns: [q | k | v] each [C, C]; head h uses cols h*HS:(h+1)*HS
    wq = w_attn[:, 0:C].reshape(C, H, HS) * scale
    wk = w_attn[:, C : 2 * C].reshape(C, H, HS)
    wv = w_attn[:, 2 * C : 3 * C].reshape(C, H, HS)
    bq = b_attn[0:C].reshape(H, HS) * scale
    bk = b_attn[C : 2 * C].reshape(H, HS)
    bv = b_attn[2 * C :].reshape(H, HS)
    wp = w_proj.reshape(H, HS, C)
    if fp8_qk:
        # fp8 path: no 1/sqrt(hs) fold (it moves into the exp scale);
        # scale w,b by 16 to center values in e4m3's normal range.
        xT8 = np.ascontiguousarray(x.reshape(B * T, C).T.astype(f8))
        wq8 = (w_attn[:, 0:C].reshape(C, H, HS) * 16.0).astype(f8)
        wk8 = (wk * 16.0).astype(f8)
        bq = b_attn[0:C].reshape(H, HS) * 16.0
        bk_ = b_attn[C : 2 * C].reshape(H, HS) * 16.0
    else:
        bk_ = bk

    in_maps = []
    for core in range(NCORES):
        hs_ = slice(HPC * core, HPC * core + HPC)
        w_qkv = np.concatenate(
            [
                wq[:, hs_, :].reshape(C, DC),
                wk[:, hs_, :].reshape(C, DC),
                wv[:, hs_, :].reshape(C, DC),
            ],
            axis=1,
        ).astype(bf)
        b_qkv = np.concatenate(
            [
                bq[hs_].reshape(DC),
                bk_[hs_].reshape(DC),
                bv[hs_].reshape(DC),
            ]
        ).astype(np.float32)
        wp_core = np.ascontiguousarray(wp[hs_].reshape(DC, C).astype(bf))
        m = {
            "xT": xT,
            "w_qkv": np.ascontiguousarray(w_qkv),
            "b_qkv": np.ascontiguousarray(b_qkv),
            "w_proj": wp_core,
        }
        if fp8_qk:
            m["xT8"] = xT8
            m["w_qk8"] = np.ascontiguousarray(
                np.concatenate(
                    [wq8[:, hs_, :].reshape(C, DC),
                     wk8[:, hs_, :].reshape(C, DC)],
                    axis=1,
                )
            )
        in_maps.append(m)
    return in_maps


def kernel(x, w_attn, b_attn, w_proj, b_proj, _trace=False):
    x = np.asarray(x, dtype=np.float32)
    w_attn = np.asarray(w_attn, dtype=np.float32)
    b_attn = np.asarray(b_attn, dtype=np.float32)
    w_proj = np.asarray(w_proj, dtype=np.float32)
    b_proj = np.asarray(b_proj, dtype=np.float32)

    B, T, C = x.shape
    H = 16
    nc = _get_nc((B, T, C, H))
    in_maps = make_in_maps(
        x, w_attn, b_attn, w_proj, B, T, C, H,
        fp8_qk=KERNEL_KWARGS.get("fp8_qk", False),
    )
    res = run_bass_kernel_spmd(
        nc, in_maps, list(range(NCORES)), trace=_trace
    )
    partials = np.stack(
        [res.results[c]["outp"].astype(np.float32) for c in range(NCORES)]
    )
    out = partials.sum(axis=0) + b_proj[None, :]
    if _trace:
        return out.reshape(B, T, C), res
    return out.reshape(B, T, C)



# revision 30
# speedup vs baseline: 1.1785x; 1.1785x over previous
"""Causal self-attention, tensor-parallel over heads across 8 NeuronCores.

Reference computation (per problem):
    qkv = x @ w_attn + b_attn ; split q,k,v ; per-head causal softmax attention
    y = att @ v ; out = y @ w_proj + b_proj
Shapes: x [4, 2048, 1024], H=16 heads, head_size=64.

Sharding: 2 heads per core (tensor parallel). Each core computes
    qkv for its heads, causal attention, and a partial y @ w_proj[rows].
Host sums the 8 partial outputs and adds b_proj (the TP all-reduce,
done host-side during the gather step).

Per-core kernel v3 (all matmuls bf16 with fp32 PSUM accumulate):
  - x passed pre-transposed (xT [C, B*T]); q pre-scaled by 1/sqrt(hs).
  - Software-pipelined schedule: batch b+1's qkv projection + v
    transposition is chopped into small quanta (a Python generator) and
    interleaved into batch b's attention j-tile loop, so TensorE never
    waits for ScalarE's exp backlog.
  - Phase 2 per (b, qg): S^T j-tiles via K=64 matmuls TWO tiles ahead
    of the attention-value matmuls, exp on ScalarE over both heads'
    PSUM banks, single 0/1 mask multiply covering both heads on the
    diagonal tiles, y^T[65, 512] accumulated with a ones-column
    denominator row.
  - Normalize: DVE reciprocal of denom row -> f32r, GpSimd
    partition_broadcast to 64 partitions, DVE multiply into packed
    yp [128, NT] (h0 rows 0-63, h1 rows 64-127, partition-shifted).
  - Phase 3: K=128 matmuls (both heads at once): out[tok, :] =
    yp.T @ w_proj_local; evictions on ScalarE (A/B-tested best).
"""

import math
import numpy as np

try:
    import concourse.bass as bass
except ImportError:  # pragma: no cover
    import sys

    sys.path.insert(0, "/opt/trn_rl_repo")
    import concourse.bass as bass

import ml_dtypes
import concourse.mybir as mybir
import concourse.tile as tile
from concourse import bacc
from concourse.bass_utils import run_bass_kernel_spmd
from concourse.masks import make_identity

BF16 = mybir.dt.bfloat16
F32 = mybir.dt.float32
F32R = mybir.dt.float32r

NCORES = 8

Exp = mybir.ActivationFunctionType.Exp
Identity = mybir.ActivationFunctionType.Identity
Copy = mybir.ActivationFunctionType.Copy


def build_nc(B=4, T=2048, C=1024, H=16, bass_kwargs=None, repeats=1,
             qk_act=False, ob_act_mod=1, p3_delay=1, ob_act_mod_tail=None,
             p3_hold=None, pd=3, fp8_qk=False, mask_pool=False,
             vt_dma=False, ob_dma_act=False):
    HS = C // H          # 64 head size
    HPC = H // NCORES    # 2 heads per core
    DC = HPC * HS        # 128 local channels
    NT = B * T           # tokens
    CK = C // 128        # contraction chunks for qkv
    QG = 512             # q-group width
    NQG = T // QG        # q-groups per batch
    NJT = T // 128       # 128-wide key tiles per batch
    JT_PER_QG = QG // 128

    assert DC == 128 and HPC == 2 and NQG == 4

    kw = dict(target_bir_lowering=False, debug=False)
    kw.update(bass_kwargs or {})
    nc = bacc.Bacc("TRN2", **kw)
    xT = nc.dram_tensor("xT", [C, NT], BF16, kind="ExternalInput")
    w_qkv = nc.dram_tensor("w_qkv", [C, 3 * DC], BF16, kind="ExternalInput")
    b_qkv = nc.dram_tensor("b_qkv", [3 * DC], F32, kind="ExternalInput")
    w_proj = nc.dram_tensor("w_proj", [DC, C], BF16, kind="ExternalInput")
    outp = nc.dram_tensor("outp", [NT, C], BF16, kind="ExternalOutput")
    if fp8_qk:
        F8 = mybir.dt.float8e4
        xT8 = nc.dram_tensor("xT8", [C, NT], F8, kind="ExternalInput")
        w_qk8 = nc.dram_tensor("w_qk8", [C, 2 * DC], F8, kind="ExternalInput")

    with tile.TileContext(nc) as tc, tc.tile_pool(name="singles", bufs=1) as sg:
        # ---- persistent SBUF ----
        qT_sb = sg.tile([128, NT], BF16)   # rows 0-63 h0, 64-127 h1
        kT_sb = sg.tile([128, NT], BF16)
        # v in natural layout, one [128, 65] tile per (b, h, j-tile);
        # col 64 is the ones column (softmax denominator trick).
        # Inner pitch is 96 (not 65): the HWDGE DMA transpose writes in
        # 32x32 tiles and silently corrupts non-32-element-aligned dest
        # offsets (verified on HW via dmat_probe.py), so every (b,h,jt)
        # tile must start 32-aligned. The matmul reads cols 0:65.
        VP = 96
        v_sb = sg.tile([128, B, HPC, NJT, VP], BF16)
        yp_sb = sg.tile([128, NT], BF16)   # packed: h0 rows 0-63, h1 64-127
        w_sb = sg.tile([128, CK, 3 * DC], BF16)
        wp_sb = sg.tile([128, C], BF16)    # both heads' w_proj rows
        bias_sb = sg.tile([128, 3], F32)
        masks_sb = sg.tile([128, JT_PER_QG, HPC, QG], BF16)
        ident_sb = sg.tile([128, 128], BF16)

        if fp8_qk:
            w8_sb = sg.tile([128, CK, 2 * DC], mybir.dt.float8e4)

        # ---- setup, ordered by first use (w chunks -> bias -> w_proj) ----
        w_r = w_qkv.rearrange("(ck p) m -> p ck m", p=128)
        ccs = [2] if fp8_qk else range(3)
        if fp8_qk:
            w8_r = w_qk8.rearrange("(ck p) m -> p ck m", p=128)
            nc.sync.dma_start(out=w8_sb, in_=w8_r)
        for cc in ccs:
            nc.sync.dma_start(
                out=w_sb[:, :, 128 * cc : 128 * cc + 128],
                in_=w_r[:, :, 128 * cc : 128 * cc + 128],
            )
        nc.sync.dma_start(
            out=bias_sb, in_=b_qkv.rearrange("(c p) -> p c", p=128)
        )
        nc.scalar.dma_start(out=wp_sb, in_=w_proj[:, :])
        make_identity(nc, ident_sb)
        # ones column of every v tile, one strided memset
        nc.vector.memset(v_sb[:, :, :, :, 64:65], 1.0)
        # mask[s][j, q] = 1.0 if q >= 128*s + j else 0  (causal, diag tiles)
        if not mask_pool:
            for s in range(JT_PER_QG):
                for h in range(HPC):
                    nc.gpsimd.memset(masks_sb[:, s, h, :], 1.0)
                    nc.gpsimd.affine_select(
                        out=masks_sb[:, s, h, :],
                        in_=masks_sb[:, s, h, :],
                        compare_op=mybir.AluOpType.is_ge,
                        fill=0.0,
                        base=-128 * s,
                        pattern=[[1, QG]],
                        channel_multiplier=-1,
                    )

        with (
            tc.tile_pool(name="xt_pool", bufs=4) as xt_pool,
            tc.tile_pool(name="xt8_pool", bufs=4) as xt8_pool,
            tc.tile_pool(name="vt_pool", bufs=2) as vt_pool,
            tc.tile_pool(name="pt_pool", bufs=4) as pt_pool,
            tc.tile_pool(name="rd_pool", bufs=2) as rd_pool,
            tc.tile_pool(name="bc_pool", bufs=2) as bc_pool,
            tc.tile_pool(name="ob_pool", bufs=4) as ob_pool,
            tc.tile_pool(name="ps_stage", bufs=2, space="PSUM") as ps_stage,
            tc.tile_pool(name="ps_yt", bufs=2, space="PSUM") as ps_yt,
            tc.tile_pool(name="ps_misc", bufs=2, space="PSUM") as ps_misc,
        ):
            xT_r = xT.rearrange("(ck p) n -> p ck n", p=128)
            if fp8_qk:
                xT8_r = xT8.rearrange("(ck p) n -> p ck n", p=128)
                F8 = mybir.dt.float8e4
                DR = mybir.MatmulPerfMode.DoubleRow

            done_groups = {}

            def gen_p1(b):
                """Generator: qkv + v-transpose for batch b, in small PE
                quanta (one yield per ~2-4 matmuls). Updates done_groups[b]
                after each 512-token group completes. Batch 0's x loads go
                out on the (startup-idle) DVE queue so they overlap the
                weight DMAs on the sync queue."""
                done_groups[b] = 0
                xts = []
                xt8s = []
                for g in range(NQG):
                    off = b * T + QG * g
                    xt = xt_pool.tile([128, CK, 512], BF16, tag="xt")
                    # batch 0 group 0 rides the startup-idle ACT queue so it
                    # overlaps the weight DMAs on the sync queue
                    dma_eng = nc.scalar if (b == 0 and g == 0) else nc.sync
                    dma_eng.dma_start(
                        out=xt, in_=xT_r[:, :, off : off + 512]
                    )
                    xts.append(xt)
                    if fp8_qk:
                        xt8 = xt8_pool.tile([128, CK, 512], F8, tag="xt8")
                        dma_eng.dma_start(
                            out=xt8, in_=xT8_r[:, :, off : off + 512]
                        )
                        xt8s.append(xt8)
                yield
                for g in range(NQG):
                    off = b * T + QG * g
                    vtmp = vt_pool.tile([128, 512], BF16, tag="vt")
                    for cc in range(3):
                        qkv_ps = ps_misc.tile([128, 512], F32, tag="misc")
                        if fp8_qk and cc < 2:
                            for c2 in range(CK // 2):
                                nc.tensor.matmul(
                                    qkv_ps,
                                    w8_sb[:, 2 * c2 : 2 * c2 + 2,
                                          128 * cc : 128 * cc + 128],
                                    xt8s[g][:, 2 * c2 : 2 * c2 + 2, :],
                                    start=(c2 == 0),
                                    stop=(c2 == CK // 2 - 1),
                                    perf_mode=DR,
                                )
                                if c2 == 1:
                                    yield
                        else:
                            for k in range(CK):
                                nc.tensor.matmul(
                                    qkv_ps,
                                    w_sb[:, k, 128 * cc : 128 * cc + 128],
                                    xts[g][:, k, :],
                                    start=(k == 0),
                                    stop=(k == CK - 1),
                                )
                                if k == 3:
                                    yield
                        if cc < 2:
                            dest = qT_sb if cc == 0 else kT_sb
                            if qk_act:
                                nc.scalar.activation(
                                    out=dest[:, off : off + 512],
                                    in_=qkv_ps,
                                    func=Identity,
                                    bias=bias_sb[:, cc : cc + 1],
                                )
                            else:
                                nc.vector.tensor_scalar_add(
                                    out=dest[:, off : off + 512],
                                    in0=qkv_ps,
                                    scalar1=bias_sb[:, cc : cc + 1],
                                )
                        else:
                            nc.vector.tensor_scalar_add(
                                out=vtmp,
                                in0=qkv_ps,
                                scalar1=bias_sb[:, 2:3],
                            )
                        yield
                    for s in range(JT_PER_QG):
                        jt = JT_PER_QG * g + s
                        for h in range(HPC):
                            if vt_dma:
                                # HWDGE 16-bit transpose: SBUF->SBUF, no
                                # PE transpose + no DVE PSUM eviction
                                nc.sync.dma_start_transpose(
                                    out=v_sb[:, b, h, jt, 0:64],
                                    in_=vtmp[64 * h : 64 * h + 64,
                                             128 * s : 128 * s + 128],
                                )
                                continue
                            vt_ps = ps_misc.tile([128, 64], BF16, tag="misc")
                            nc.tensor.transpose(
                                vt_ps,
                                vtmp[64 * h : 64 * h + 64,
                                     128 * s : 128 * s + 128],
                                ident_sb[64 * h : 64 * h + 64,
                                         64 * h : 64 * h + 64],
                            )
                            nc.vector.tensor_copy(
                                out=v_sb[:, b, h, jt, 0:64], in_=vt_ps
                            )
                        yield
                    done_groups[b] = g + 1

            fillers = []

            def fill(n=1):
                for _ in range(n):
                    while fillers:
                        if next(fillers[0], StopIteration) is StopIteration:
                            fillers.pop(0)
                        else:
                            break
                    if not fillers:
                        return

            def fill_until_group(b, g):
                while done_groups.get(b, 0) < g and fillers:
                    fill(1)

            pre_norm_hook = [None]

            def emit_p2(b, qg):
                """Causal attention for (b, q-group); writes yp_sb columns."""
                q0 = b * T + QG * qg
                njt = JT_PER_QG * (qg + 1)
                yts = [
                    ps_yt.tile([65, 512], F32, tag="yt", name=f"yt{h}")
                    for h in range(HPC)
                ]

                def col_lo(jt):
                    # diagonal sub-tile s: columns [0, 128*s) are entirely
                    # non-causal -- skip them in S / exp / mask / y
                    s = jt - JT_PER_QG * qg
                    return 128 * s if s > 0 else 0

                def emit_S(jt):
                    lo = col_lo(jt)
                    st = ps_stage.tile([128, 2, 512], F32, tag="stage")
                    for h in range(HPC):
                        hl = slice(64 * h, 64 * h + 64)
                        nc.tensor.matmul(
                            st[:, h, lo:QG],
                            kT_sb[hl, b * T + 128 * jt : b * T + 128 * jt + 128],
                            qT_sb[hl, q0 + lo : q0 + QG],
                            start=True,
                            stop=True,
                        )
                    pt = pt_pool.tile([128, 2, 512], BF16, tag="pt")
                    # fp8_qk: q,k were computed from host-prescaled (x16)
                    # fp8 weights+biases; fold 1/(16*16) and the 1/sqrt(hs)
                    # attention scale into the exp activation's scale.
                    nc.scalar.activation(
                        out=pt[:, :, lo:QG], in_=st[:, :, lo:QG], func=Exp,
                        scale=(0.125 / 256.0) if fp8_qk else 1.0,
                    )
                    if jt >= JT_PER_QG * qg:  # diagonal tile: causal zeroing
                        s = jt - JT_PER_QG * qg
                        if mask_pool:
                            # keep pt[j, h, q] where (lo+qr) >= 128*s + j;
                            # lo == 128*s, so base = 0. Runs on the
                            # (otherwise idle) Pool engine instead of DVE.
                            nc.gpsimd.affine_select(
                                out=pt[:, :, lo:QG],
                                in_=pt[:, :, lo:QG],
                                compare_op=mybir.AluOpType.is_ge,
                                fill=0.0,
                                base=0,
                                pattern=[[0, HPC], [1, QG - lo]],
                                channel_multiplier=-1,
                            )
                        else:
                            nc.vector.tensor_mul(
                                pt[:, :, lo:QG],
                                pt[:, :, lo:QG],
                                masks_sb[:, s, :, lo:QG],
                            )
                    return pt

                def emit_y(pt, jt):
                    lo = col_lo(jt)
                    for h in range(HPC):
                        nc.tensor.matmul(
                            yts[h][:, lo:QG],
                            v_sb[:, b, h, jt, 0:65],
                            pt[:, h, lo:QG],
                            start=(jt == 0),
                            stop=(jt == njt - 1),
                        )

                pts = {}
                for jt in range(njt):
                    pts[jt] = emit_S(jt)
                    fill(1)
                    if jt >= pd:
                        emit_y(pts.pop(jt - pd), jt - pd)
                        fill(1)
                for jt in range(max(0, njt - pd), njt):
                    emit_y(pts.pop(jt), jt)
                    if jt < njt - 1:
                        fill(1)
                if pre_norm_hook[0] is not None:
                    hook, pre_norm_hook[0] = pre_norm_hook[0], None
                    hook()

                # normalize: yp[:, q] = y[:, q] / den[q]
                for h in range(HPC):
                    rd = rd_pool.tile([1, 512], F32R, tag="rd")
                    with nc.allow_low_precision("f32r denom recip"):
                        nc.vector.reciprocal(rd, yts[h][64:65, :])
                    bc_sb = bc_pool.tile([64, 512], F32R, tag="bc")
                    nc.gpsimd.partition_broadcast(bc_sb, rd)
                    nc.vector.tensor_mul(
                        out=yp_sb[64 * h : 64 * h + 64, q0 : q0 + QG],
                        in0=yts[h][0:64, :],
                        in1=bc_sb,
                    )

            def emit_p3(b, qg, cur_b):
                """out[tok, :] = yp.T @ w_proj for this q-group's tokens.
                cur_b: the batch whose attention loop we're emitting inside
                (controls which engine takes the eviction)."""
                mod = ob_act_mod
                if ob_act_mod_tail is not None and cur_b == B - 1:
                    mod = ob_act_mod_tail
                for i in range(QG // 128):
                    t0 = b * T + QG * qg + 128 * i
                    pr = ps_stage.tile([128, 2, 512], F32, tag="stage")
                    for ns in range(2):
                        nc.tensor.matmul(
                            pr[:, ns, :],
                            yp_sb[:, t0 : t0 + 128],
                            wp_sb[:, 512 * ns : 512 * ns + 512],
                            start=True,
                            stop=True,
                        )
                    ob = ob_pool.tile([128, 1024], BF16, tag="ob")
                    if mod == -1:  # split halves across ACT and DVE
                        nc.scalar.activation(
                            out=ob[:, 0:512], in_=pr[:, 0, :], func=Copy
                        )
                        nc.vector.tensor_copy(
                            out=ob[:, 512:1024], in_=pr[:, 1, :]
                        )
                    elif mod and i % mod == 0:
                        nc.scalar.activation(out=ob, in_=pr, func=Copy)
                    else:
                        nc.vector.tensor_copy(out=ob, in_=pr)
                    ob_eng = nc.scalar if ob_dma_act else nc.sync
                    ob_eng.dma_start(out=outp[t0 : t0 + 128, :], in_=ob)
                    fill(1)

            for _rep in range(repeats):
                # Progressive prologue: attention for (b, qg) starts as soon
                # as batch b's qkv groups 0..qg are emitted; the rest of the
                # qkv work (and the next batch's) rides along as filler
                # inside the attention loops. P3 runs one q-group late so
                # the normalize chain overlaps the next q-group's S matmuls.
                hold = p3_hold or {}
                p3_pending = []
                fillers.clear()
                done_groups.clear()
                fillers.append(gen_p1(0))
                for b in range(B):
                    if b + 1 < B:
                        fillers.append(gen_p1(b + 1))
                    # last batch: largest q-group first, so the run ends on
                    # the smallest exp backlog
                    qgs = range(NQG) if b + 1 < B else range(NQG - 1, -1, -1)
                    for qg in qgs:
                        fill_until_group(b, qg + 1)
                        emit_p2(b, qg)
                        p3_pending.append((b, qg))
                        while len(p3_pending) > hold.get(b, p3_delay):
                            emit_p3(*p3_pending.pop(0), cur_b=b)
                fill(10**6)
                while p3_pending:
                    emit_p3(*p3_pending.pop(0), cur_b=B - 1)

    nc.compile()
    return nc


_NC_CACHE = {}

# variant knobs used by kernel() (and test.py).
# vt_dma: v-transpose via the HWDGE 16-bit DMA transpose instead of PE
# transpose + DVE PSUM eviction. Requires the 32-element-aligned v_sb
# pitch (VP=96): the DMA transpose writes 32x32 tiles and corrupts
# non-32-aligned dest offsets (found via dmat_probe.py on HW). With the
# aligned pitch it is bit-identical to the PE path on device
# (rel l2 0.005496) and measured -10% marginal HW time in a clean
# reps-65 paired A/B (371.6us -> 333.3us).
KERNEL_KWARGS = {"vt_dma": True}


def _get_nc(shape_key):
    if shape_key not in _NC_CACHE:
        _NC_CACHE[shape_key] = build_nc(*shape_key, **KERNEL_KWARGS)
    return _NC_CACHE[shape_key]


def make_in_maps(x, w_attn, b_attn, w_proj, B, T, C, H, fp8_qk=False):
    HS = C // H
    HPC = H // NCORES
    DC = HPC * HS
    scale = 1.0 / math.sqrt(HS)
    bf = ml_dtypes.bfloat16
    f8 = ml_dtypes.float8_e4m3

    xT = np.ascontiguousarray(
        x.reshape(B * T, C).T.astype(bf)
    )
    # w_attn columns: [q | k | v] each [C, C]; head h uses cols h*HS:(h+1)*HS
    wq = w_attn[:, 0:C].reshape(C, H, HS) * scale
    wk = w_attn[:, C : 2 * C].reshape(C, H, HS)
    wv = w_attn[:, 2 * C : 3 * C].reshape(C, H, HS)
    bq = b_attn[0:C].reshape(H, HS) * scale
    bk = b_attn[C : 2 * C].reshape(H, HS)
    bv = b_attn[2 * C :].reshape(H, HS)
    wp = w_proj.reshape(H, HS, C)
    if fp8_qk:
        # fp8 path: no 1/sqrt(hs) fold (it moves into the exp scale);
        # scale w,b by 16 to center values in e4m3's normal range.
        xT8 = np.ascontiguousarray(x.reshape(B * T, C).T.astype(f8))
        wq8 = (w_attn[:, 0:C].reshape(C, H, HS) * 16.0).astype(f8)
        wk8 = (wk * 16.0).astype(f8)
        bq = b_attn[0:C].reshape(H, HS) * 16.0
        bk_ = b_attn[C : 2 * C].reshape(H, HS) * 16.0
    else:
        bk_ = bk

    in_maps = []
    for core in range(NCORES):
        hs_ = slice(HPC * core, HPC * core + HPC)
        w_qkv = np.concatenate(
            [
                wq[:, hs_, :].reshape(C, DC),
                wk[:, hs_, :].reshape(C, DC),
                wv[:, hs_, :].reshape(C, DC),
            ],
            axis=1,
        ).astype(bf)
        b_qkv = np.concatenate(
            [
                bq[hs_].reshape(DC),
                bk_[hs_].reshape(DC),
                bv[hs_].reshape(DC),
            ]
        ).astype(np.float32)
        wp_core = np.ascontiguousarray(wp[hs_].reshape(DC, C).astype(bf))
        m = {
            "xT": xT,
            "w_qkv": np.ascontiguousarray(w_qkv),
            "b_qkv": np.ascontiguousarray(b_qkv),
            "w_proj": wp_core,
        }
        if fp8_qk:
            m["xT8"] = xT8
            m["w_qk8"] = np.ascontiguousarray(
                np.concatenate(
                    [wq8[:, hs_, :].reshape(C, DC),
                     wk8[:, hs_, :].reshape(C, DC)],
                    axis=1,
                )
            )
        in_maps.append(m)
    return in_maps


def kernel(x, w_attn, b_attn, w_proj, b_proj, _trace=False):
    x = np.asarray(x, dtype=np.float32)
    w_attn = np.asarray(w_attn, dtype=np.float32)
    b_attn = np.asarray(b_attn, dtype=np.float32)
    w_proj = np.asarray(w_proj, dtype=np.float32)
    b_proj = np.asarray(b_proj, dtype=np.float32)

    B, T, C = x.shape
    H = 16
    nc = _get_nc((B, T, C, H))
    in_maps = make_in_maps(
        x, w_attn, b_attn, w_proj, B, T, C, H,
        fp8_qk=KERNEL_KWARGS.get("fp8_qk", False),
    )
    res = run_bass_kernel_spmd(
        nc, in_maps, list(range(NCORES)), trace=_trace
    )
    partials = np.stack(
        [res.results[c]["outp"].astype(np.float32) for c in range(NCORES)]
    )
    out = partials.sum(axis=0) + b_proj[None, :]
    if _trace:
        return out.reshape(B, T, C), res
    return out.reshape(B, T, C)

